# revision 1
# baseline (speedup 1.0000x reference)
"""Trainium2 Bass kernel for nn_Colorizer (retrieval_knn).

Computation (per reference frame r of 3, for each pixel p of a 128x128 image):
  corr[r, n, p] = <feats_t[:, p], feats_r[r, :, p + offset(n)]>   n in 13x13 window
  q_val[r, p]  = max_n corr ; q_idx[r, p] = argmax_n corr (first occurrence)
  gathered[r, c, p] = quantized_sub[r, c, p + offset(q_idx)]      (zero padded)
  out[c, p] = sum_r softmax_r(q_val)[r] * gathered[r, c, p]

Sharding: the spatial h dim is split into 8 bands of 16 rows (one per core);
each core handles all 3 refs for its band, so the softmax over refs is local
and no device collective is needed.  Host reassembles the row bands.

Device algorithm per core: for each tile of 128 pixels (16 rows x 8 cols) the
TensorEngine computes the Gram matrix between the tile's feats_t vectors
(lhsT, c=128 contraction) and the 28x20 zero-padded feats_r halo window
(rhs, 560 columns -> 2 PSUM banks).  A constant additive mask (-1e30 at
window positions outside a pixel's own 13x13 patch) turns per-pixel argmax
over the shared window into a plain free-dim argmax: one fused DVE
tensor_tensor_reduce (mask add + max accumulate) and one max_index.  The
argmax index is decoded to a row offset into the padded, channel-last
subsampled quantized_r image and the 3-channel pixel is fetched with one
indirect DMA gather.  A tiny fp32 softmax over the 3 refs weights the
gathered values.
"""

import os

import numpy as np

import concourse.bass as bass
import concourse.mybir as mybir
import concourse.tile as tile
from concourse import bacc
from concourse.bass import IndirectOffsetOnAxis
from concourse.bass_utils import run_bass_kernel_spmd

F32 = mybir.dt.float32
F32R = mybir.dt.float32r
BF16 = mybir.dt.bfloat16
U16 = mybir.dt.uint16
U32 = mybir.dt.uint32
I32 = mybir.dt.int32

NCORES = 8
NREF, C, H, W = 3, 128, 128, 128
RAD = 6                      # patch radius
PS = 2 * RAD + 1             # 13
CQ = 3                       # quantized channels
SUB = 4                      # quantized_r spatial subsample stride

ROWS = H // NCORES           # 16 rows per core
XB = 8                       # x block size
NT = W // XB                 # 16 tiles per ref
WY = ROWS + 2 * RAD          # 28 window rows
WX = XB + 2 * RAD            # 20 window cols
WIN = WY * WX                # 560
HALF = WY // 2               # 14 window rows per PSUM bank
NHALF = HALF * WX            # 280 columns per matmul
PW = W + 2 * RAD             # 140 padded width
NRT = NREF * NT              # 48 (ref, tile) pairs
NEG = -1.0e30

_CACHE: dict = {}


def _max_index_raw(nc, out, in_max, in_values):
    """max_index accepting a multi-dim in_values AP (e.g. a two-bank PSUM
    view); the bass wrapper's 2-D assert is stricter than the hardware."""
    eng = nc.vector
    return eng.add_instruction(
        mybir.InstMaxIndex(
            name=nc.get_next_instruction_name(),
            ins=[eng.lower_ap(in_max), eng.lower_ap(in_values)],
            outs=[eng.lower_ap(out)],
        )
    )


def _build_program(stage: int = 3) -> bacc.Bacc:
    """stage 1: corr+argmax only; 2: +gathers; 3: full (softmax combine)."""
    nc = bacc.Bacc("TRN2", target_bir_lowering=False, debug=False)

    ft_d = nc.dram_tensor("ft", [C, ROWS * W], F32, kind="ExternalInput")
    frp_d = nc.dram_tensor("frp", [NREF, C, WY * PW], F32, kind="ExternalInput")
    # one [WIN, CQ] window table per (ref, tile): the raw argmax index is
    # the gather row, no on-device index arithmetic needed
    qrp_d = [
        nc.dram_tensor(f"qrp{rt}", [WIN, CQ], F32, kind="ExternalInput")
        for rt in range(NRT)
    ]
    mask_d = nc.dram_tensor("mask", [128, WIN], BF16, kind="ExternalInput")
    ident_d = nc.dram_tensor("ident", [128, 128], BF16, kind="ExternalInput")
    # raw layout [pixel_partition=(yl,xl), tile, channel]; host untangles
    out_d = nc.dram_tensor("out", [128, NT * CQ], F32, kind="ExternalOutput")

    with tile.TileContext(nc) as tc:
        with (
            tc.tile_pool(name="const", bufs=1) as constp,
            tc.tile_pool(name="psum", bufs=4, space="PSUM") as psump,
            tc.tile_pool(name="small", bufs=1) as smallp,
        ):
            # split the startup loads so tile (r0,t0) can begin before the
            # full 7MB of inputs lands: ft block 0 + the left half of frp0
            ft_sb = constp.tile([C, ROWS * W], F32, tag="ft")
            nc.sync.dma_start(out=ft_sb[:, 0:512], in_=ft_d.ap()[:, 0:512])
            frp_sb = []
            for r in range(NREF):
                t_ = constp.tile([C, WY * PW], F32, tag=f"frp{r}")
                frp_sb.append(t_)
            fr0v = frp_sb[0][:].rearrange("c (y x) -> c y x", x=PW)
            fr0d = frp_d.ap()[0].rearrange("c (y x) -> c y x", x=PW)
            nc.sync.dma_start(out=fr0v[:, :, 0:76], in_=fr0d[:, :, 0:76])
            nc.sync.dma_start(out=ft_sb[:, 512:], in_=ft_d.ap()[:, 512:])
            nc.sync.dma_start(out=fr0v[:, :, 76:PW], in_=fr0d[:, :, 76:PW])
            for r in range(1, NREF):
                nc.sync.dma_start(out=frp_sb[r][:], in_=frp_d.ap()[r])
            mask_sb = constp.tile([128, WIN], BF16, tag="mask")
            nc.sync.dma_start(out=mask_sb[:], in_=mask_d.ap())
            ident_sb = constp.tile([128, 128], BF16, tag="ident")
            nc.sync.dma_start(out=ident_sb[:], in_=ident_d.ap())

            max_sb = smallp.tile([128, NRT * 8], F32, tag="max8")
            idx_sb = smallp.tile([128, NRT * 8], U32, tag="idx")
            gath = smallp.tile([128, NRT * CQ], F32, tag="gath")
            gathv = gath[:].rearrange("p (s c) -> p s c", c=CQ)

            # ft is host-arranged as [c, (t, yl, xl)]: tile t's 128 pixels
            # are contiguous (walrus requires a 1-free-dim weights AP).
            # r-outer so ref 0's compute overlaps refs 1/2 input DMA.
            for r in range(NREF):
                frv = frp_sb[r][:].rearrange("c (y x) -> c y x", x=PW)
                for t in range(NT):
                    rt = r * NT + t
                    ps = psump.tile([128, 1024], F32, tag="ps")
                    lhsT = ft_sb[:, t * 128 : (t + 1) * 128]
                    rhs1 = frv[:, 0:HALF, t * XB : t * XB + WX]
                    rhs2 = frv[:, HALF:WY, t * XB : t * XB + WX]
                    # corr Gram into two PSUM banks, then accumulate the
                    # -1e30 valid-window mask via an identity matmul
                    nc.tensor.matmul(
                        ps[:, 0:NHALF], lhsT, rhs1, start=True, stop=False
                    )
                    nc.tensor.matmul(
                        ps[:, 512 : 512 + NHALF], lhsT, rhs2, start=True, stop=False
                    )
                    nc.tensor.matmul(
                        ps[:, 0:NHALF],
                        ident_sb[:],
                        mask_sb[:, 0:NHALF],
                        start=False,
                        stop=True,
                    )
                    nc.tensor.matmul(
                        ps[:, 512 : 512 + NHALF],
                        ident_sb[:],
                        mask_sb[:, NHALF:WIN],
                        start=False,
                        stop=True,
                    )
                    psv = ps[:].rearrange("p (b n) -> p b n", b=2)[:, :, 0:NHALF]
                    nc.vector.max(
                        out=max_sb[:, rt * 8 : (rt + 1) * 8], in_=psv
                    )
                    _max_index_raw(
                        nc,
                        idx_sb[:, rt * 8 : (rt + 1) * 8],
                        max_sb[:, rt * 8 : (rt + 1) * 8],
                        psv,
                    )
                    if stage < 2:
                        continue
                    # gather the argmax patch pixel (3 ch) for this tile;
                    # issued inline so it hides under later tiles' compute
                    nc.gpsimd.indirect_dma_start(
                        out=gathv[:, rt],
                        out_offset=None,
                        in_=qrp_d[rt].ap(),
                        in_offset=IndirectOffsetOnAxis(
                            ap=idx_sb[:, rt * 8 : rt * 8 + 1], axis=0
                        ),
                    )

            # ---- softmax over the 3 refs, weighted sum of gathered pixels ----
            qve = max_sb[:].rearrange("p (s e) -> p s e", e=8)
            if stage < 3:
                # debug stages: just dump qval (and gathers ran if stage 2)
                oacc = smallp.tile([128, NT * CQ], F32, tag="oacc")
                nc.vector.tensor_copy(
                    out=oacc[:, 0:NRT].rearrange("p (s o) -> p s o", o=1),
                    in_=qve[:, :, 0:1],
                )
                nc.sync.dma_start(out=out_d.ap(), in_=oacc[:])
            if stage >= 3:
                _softmax_combine(nc, smallp, qve, gath, out_d)

    nc.compile()
    return nc


def _softmax_combine(nc, smallp, qve, gath, out_d):
    # qve: [128, NRT, 8] view of the top-8 accumulator; col 0 = q_val
    qv = [qve[:, r * NT : (r + 1) * NT, 0] for r in range(NREF)]
    m01 = smallp.tile([128, NT], F32, tag="m01")
    nc.vector.tensor_tensor(
        out=m01[:], in0=qv[0], in1=qv[1], op=mybir.AluOpType.max
    )
    mm = smallp.tile([128, NT], F32, tag="mm")
    nc.vector.tensor_tensor(
        out=mm[:], in0=m01[:], in1=qv[2], op=mybir.AluOpType.max
    )
    es = []
    for r in range(NREF):
        e_ = smallp.tile([128, NT], F32, tag=f"e{r}")
        nc.vector.tensor_tensor(
            out=e_[:], in0=qv[r], in1=mm[:], op=mybir.AluOpType.subtract
        )
        nc.scalar.activation(
            out=e_[:], in_=e_[:], func=mybir.ActivationFunctionType.Exp
        )
        es.append(e_)
    ssum = smallp.tile([128, NT], F32, tag="ssum")
    nc.vector.tensor_tensor(
        out=ssum[:], in0=es[0][:], in1=es[1][:], op=mybir.AluOpType.add
    )
    nc.vector.tensor_tensor(
        out=ssum[:], in0=ssum[:], in1=es[2][:], op=mybir.AluOpType.add
    )
    rec = smallp.tile([128, NT], F32, tag="rec")
    nc.vector.reciprocal(out=rec[:], in_=ssum[:])

    gv = gath[:].rearrange("p (r s c) -> p r s c", r=NREF, c=CQ)
    oacc = smallp.tile([128, NT * CQ], F32, tag="oacc")
    oaccv = oacc[:].rearrange("p (s c) -> p s c", c=CQ)
    for r in range(NREF):
        w_ = smallp.tile([128, NT], F32, tag=f"w{r}")
        nc.vector.tensor_tensor(
            out=w_[:], in0=es[r][:], in1=rec[:], op=mybir.AluOpType.mult
        )
        wb = w_[:].rearrange("p (s o) -> p s o", o=1).to_broadcast([128, NT, CQ])
        if r == 0:
            nc.vector.tensor_tensor(
                out=oaccv, in0=gv[:, r], in1=wb, op=mybir.AluOpType.mult
            )
        else:
            term = smallp.tile([128, NT * CQ], F32, tag=f"term{r}")
            termv = term[:].rearrange("p (s c) -> p s c", c=CQ)
            nc.vector.tensor_tensor(
                out=termv, in0=gv[:, r], in1=wb, op=mybir.AluOpType.mult
            )
            nc.vector.tensor_tensor(
                out=oaccv, in0=oaccv, in1=termv, op=mybir.AluOpType.add
            )

    nc.sync.dma_start(out=out_d.ap(), in_=oacc[:])


def _host_prep(feats_r, feats_t, quantized_r):
    """Build the 8 per-core input maps."""
    frp_full = np.zeros((NREF, C, H + 2 * RAD, PW), np.float32)
    frp_full[:, :, RAD : RAD + H, RAD : RAD + W] = feats_r[:, 0]

    qr = np.ascontiguousarray(quantized_r[:, 0, :, ::SUB, ::SUB], np.float32)
    qrp_full = np.zeros((NREF, H + 2 * RAD, PW, CQ), np.float32)
    qrp_full[:, RAD : RAD + H, RAD : RAD + W, :] = qr.transpose(0, 2, 3, 1)

    # mask[p=(yl,xl), n=(y',x')] = 0 inside pixel (yl,xl)'s own 13x13 patch
    yl = np.arange(ROWS)[:, None, None, None]
    xl = np.arange(XB)[None, :, None, None]
    yw = np.arange(WY)[None, None, :, None]
    xw = np.arange(WX)[None, None, None, :]
    valid = (
        (yw - yl >= 0) & (yw - yl < PS) & (xw - xl >= 0) & (xw - xl < PS)
    )
    import ml_dtypes

    mask = np.where(valid, 0.0, NEG).astype(ml_dtypes.bfloat16).reshape(128, WIN)
    ident = np.eye(128, dtype=np.float32).astype(ml_dtypes.bfloat16)

    in_maps = []
    for k in range(NCORES):
        y0 = ROWS * k
        # [c, yl, t, xl] -> [c, t, yl, xl]: tile-major, pixels contiguous
        ft_core = np.ascontiguousarray(
            feats_t[0][:, y0 : y0 + ROWS, :]
            .reshape(C, ROWS, NT, XB)
            .transpose(0, 2, 1, 3)
            .reshape(C, ROWS * W)
        )
        frp_core = np.ascontiguousarray(
            frp_full[:, :, y0 : y0 + WY, :].reshape(NREF, C, WY * PW)
        )
        m = {"ft": ft_core, "frp": frp_core, "mask": mask, "ident": ident}
        # per-(ref, tile) gather window table [WIN, CQ]
        qc = qrp_full[:, y0 : y0 + WY, :, :]  # [NREF, WY, PW, CQ]
        for r in range(NREF):
            for t in range(NT):
                m[f"qrp{r * NT + t}"] = np.ascontiguousarray(
                    qc[r, :, t * XB : t * XB + WX, :].reshape(WIN, CQ)
                )
        in_maps.append(m)
    return in_maps


def _install_ntff_shim():
    """This container's antenv lacks axon_hooks, so run_bass_kernel_spmd's
    trace path can't find the NTFF profile hook. Inject the module and
    register the ctypes-based hook from the boot script. Best-effort."""
    try:
        import sys
        import types

        if "antenv.axon_hooks" in sys.modules:
            return
        mod = types.ModuleType("antenv.axon_hooks")
        holder = [None]
        mod.set_axon_ntff_profile_hook = lambda h: holder.__setitem__(0, h)
        mod.get_axon_ntff_profile_hook = lambda: holder[0]
        sys.modules["antenv.axon_hooks"] = mod
        import antenv

        antenv.axon_hooks = mod
        from trn_agent_boot.trn_boot import _ntff_profile_via_ctypes

        hook = _ntff_profile_via_ctypes("/opt/axon/libaxon_pjrt.so")
        if hook is not None:
            mod.set_axon_ntff_profile_hook(hook)
    except Exception as e:  # pragma: no cover - tracing is best-effort
        print(f"ntff shim install failed: {e}")


last_exec_time_ns = None


def kernel(feats_r, feats_t, quantized_r, ref_index=None, current_ind=None):
    global last_exec_time_ns
    feats_r = np.asarray(feats_r, np.float32)
    feats_t = np.asarray(feats_t, np.float32)
    quantized_r = np.asarray(quantized_r, np.float32)

    in_maps = _host_prep(feats_r, feats_t, quantized_r)

    if "nc" not in _CACHE:
        _CACHE["nc"] = _build_program()
    nc = _CACHE["nc"]

    trace = bool(int(os.environ.get("KERNEL_TRACE", "0")))
    kwargs = {}
    if trace:
        _install_ntff_shim()
        tdir = os.environ.get("KERNEL_TRACE_DIR")
        if tdir:
            os.makedirs(tdir, exist_ok=True)
            kwargs["tmpdir"] = tdir
    res = run_bass_kernel_spmd(
        nc, in_maps, list(range(NCORES)), trace=trace, **kwargs
    )
    last_exec_time_ns = res.exec_time_ns

    out = np.concatenate(
        [_unshard_core(res.results[k]["out"]) for k in range(NCORES)], axis=1
    )
    return np.ascontiguousarray(out.reshape(1, CQ, H, W), np.float32)


def _unshard_core(raw):
    # raw [128, NT*CQ] with partition p=(yl,xl), free (t, c) -> [CQ, ROWS, W]
    r = np.asarray(raw).reshape(ROWS, XB, NT, CQ)
    return r.transpose(3, 0, 2, 1).reshape(CQ, ROWS, W)

